# revision 27
# baseline (speedup 1.0000x reference)
"""Multi-head attention Trainium2 Bass kernel.

Problem: B=4, N=M=2048, DM=512, H=8, DH=64, DO=512, fp32.
Sharding: 8 cores = (batch b, row-half) -- each core computes full attention
for 1024 query rows of one batch. No collectives.

Per-core dataflow (v4 -- oh flipped to [n, 65], bf16 attention operands,
dense PE schedule):
  - inputs stream in as 2-row-tile pair DMAs alternating the SP(HWDGE) and
    Pool(SWDGE) queues; constants arrive as one packed byte-tensor DMA
  - PE-transpose Q,K,V 128x128 blocks; transposed K/Q staging persists so
    per-head projections interleave into later attention windows
  - kTf/qTf [hdh, m|n] bf16 (bias + 1/sqrt(dh) folded host-side)
  - vha [m, h, 65] bf16 = [Vh + vb | 1]  (v-bias exact since sum(attn)=1)
  - scoresT[m, n] = kh @ qhT per head pair (tile_position row packing);
    window 0 scores interleave with the K/Q lead-in so exp starts early
  - exp on ScalarE (PSUM fp32 -> SBUF bf16)
  - oh[n, 65] = ex^T(stationary) @ vha(moving, F=65); col 64 = denominator
  - normalize on DVE: per-partition reciprocal + multiply -> mh2 bf16;
    mh transposes are deferred "finishers" so the PE never waits on DVE
  - out[n, do] = sum_hp mhT_hp^T @ wp_hp + bias (ones-row matmul)
Loop nest: hp (head pair) outer, nb (n-half) inner; window w = hp*2+nb.
V projection fills windows 0-1 (oh of w0 runs late in w1); kTf/qTf
head-pair projections fill windows 1-3; nb0 output projections fill
window 7 (ab-major scores there); nb1 outputs drain in the tail.
"""
import os
import sys

sys.path.insert(0, "/opt/trn_rl_repo")

import numpy as np
import ml_dtypes

import concourse.bass as bass
import concourse.mybir as mybir
import concourse.tile as tile
from concourse import bacc
from concourse.bass_utils import run_bass_kernel_spmd

F32 = mybir.dt.float32
F32R = mybir.dt.float32r
BF16 = mybir.dt.bfloat16
U8 = mybir.dt.uint8
EXP = mybir.ActivationFunctionType.Exp
ADD = mybir.AluOpType.add
MULT = mybir.AluOpType.mult

P = 128
DM = 512
HDH = 512
DH = 64
H = 8
NB = 1024     # query rows per core
M = 2048      # kv rows
DO = 512
N_MT = M // P
N_QT = NB // P

_CACHED = {}
LAST_EXEC_NS = None
_SECTION = None  # optional trace-attribution hook: list whose [0] is set


def _mark(s):
    if _SECTION is not None:
        _SECTION[0] = s


def _build():
    nc = bacc.Bacc("TRN2", target_bir_lowering=False, debug=False)

    d_q = nc.declare_dram_parameter("q", [NB, DM], BF16, isOutput=False)
    d_k = nc.declare_dram_parameter("k", [M, DM], BF16, isOutput=False)
    d_v = nc.declare_dram_parameter("v", [M, DM], BF16, isOutput=False)
    d_wq = nc.declare_dram_parameter("wq", [DM, HDH], BF16, isOutput=False)
    d_wk = nc.declare_dram_parameter("wk", [DM, HDH], BF16, isOutput=False)
    d_wv = nc.declare_dram_parameter("wv", [DM, HDH], BF16, isOutput=False)
    d_wp = nc.declare_dram_parameter("wp", [HDH, DO], BF16, isOutput=False)
    d_consts = nc.declare_dram_parameter("consts", [P, 264], F32R, isOutput=False)
    d_idb = nc.declare_dram_parameter("identb", [P, P], BF16, isOutput=False)
    d_vbrow = nc.declare_dram_parameter("vbrow", [1, HDH], F32R, isOutput=False)
    d_pb = nc.declare_dram_parameter("pb", [1, DO], F32R, isOutput=False)
    d_out = nc.declare_dram_parameter("out", [NB, DO], F32, isOutput=True)

    with tile.TileContext(nc) as tc:
        from contextlib import ExitStack
        with ExitStack() as ctx:
            persist = ctx.enter_context(tc.tile_pool(name="persist", bufs=1))
            raw = ctx.enter_context(tc.tile_pool(name="raw", bufs=6))
            ex_pool = ctx.enter_context(tc.tile_pool(name="expp", bufs=18))
            nm = ctx.enter_context(tc.tile_pool(name="nm", bufs=4))
            mh2_pool = ctx.enter_context(tc.tile_pool(name="mh2", bufs=3))
            ps_sc = ctx.enter_context(tc.tile_pool(name="ps_sc", bufs=3, space="PSUM"))
            ps_wk = ctx.enter_context(tc.tile_pool(name="ps_wk", bufs=2, space="PSUM"))

            # --- packed constants: one f32r DMA (+ tiny bf16 identity) ---
            consts = persist.tile([P, 264], F32R, tag="consts", name="consts")
            nc.sync.dma_start(consts[:], d_consts[:])
            identb = persist.tile([P, P], BF16, tag="identb", name="identb")
            nc.sync.dma_start(identb[:], d_idb[:])
            ident = consts[:, 0:128]
            ones = consts[:, 128:256]
            ones_f32 = consts[:, 128:256].bitcast(F32)
            qb = consts[:, 256:260].bitcast(F32)
            kb = consts[:, 260:264].bitcast(F32)

            # --- persistent tensors ---
            kTf = [persist.tile([P, M], BF16, tag=f"kTf{i}", name=f"kTf{i}")
                   for i in range(4)]
            qTf = [persist.tile([P, NB], BF16, tag=f"qTf{i}", name=f"qTf{i}")
                   for i in range(4)]
            ktsK = [persist.tile([P, 4, 512], BF16, tag=f"ktsK{i}", name=f"ktsK{i}")
                    for i in range(4)]
            ktsQ = [persist.tile([P, 4, 512], BF16, tag=f"ktsQ{i}", name=f"ktsQ{i}")
                    for i in range(2)]
            vha = persist.tile([P, N_MT, H, 65], BF16, tag="vha", name="vha")
            VT_sb = persist.tile([P, 4, M], BF16, tag="VT", name="VT")
            mhT = [[persist.tile([P, 512], BF16, tag=f"mhT{nb}_{hp}",
                                 name=f"mhT{nb}_{hp}")
                    for hp in range(4)] for nb in range(2)]
            vbb = persist.tile([P, H, DH], BF16, tag="vbb", name="vbb")
            pb = persist.tile([1, DO], F32R, tag="pb", name="pb")
            vbrow = persist.tile([1, HDH], F32R, tag="vbrow", name="vbrow")
            wk_sb = persist.tile([P, 4, HDH], BF16, tag="wk", name="wk")
            wq_sb = persist.tile([P, 4, HDH], BF16, tag="wq", name="wq")
            wv_sb = persist.tile([P, 4, HDH], BF16, tag="wv", name="wv")
            wp_sb = persist.tile([P, 4, DO], BF16, tag="wp", name="wp")

            def load_pair(d_src, t0, eng):
                """One DMA loading 2 bf16 row-tiles as [p, j, c]."""
                st = raw.tile([P, 2, DM], BF16, tag="pairb", name="pairb")
                eng.dma_start(
                    st[:],
                    d_src[t0 * P:(t0 + 2) * P, :].rearrange(
                        "(j p) c -> p j c", p=P))
                return st

            def load_w_half(w_sb, d_w, h, eng):
                eng.dma_start(
                    w_sb[:, 2 * h:2 * h + 2, :],
                    d_w[2 * h * P:(2 * h + 2) * P, :].rearrange(
                        "(j p) c -> p j c", p=P))

            def transpose_pair(st, ts, j0, dve_only=False):
                """Transpose 2 row-tiles from st into ts slices j0, j0+1."""
                _mark("in_transpose")
                for jj in range(2):
                    pst = ps_wk.tile([P, DM], F32, tag="pj",
                                     name="pj").bitcast(BF16)[:, 0:DM]
                    for dc in range(4):
                        nc.tensor.transpose(
                            pst[:, dc * P:(dc + 1) * P],
                            st[:, jj, dc * P:(dc + 1) * P], identb[:],
                        )
                    eng = (nc.vector.tensor_copy if (dve_only or jj % 2)
                           else nc.scalar.copy)
                    eng(
                        ts[:, :, (j0 + jj) * P:(j0 + jj + 1) * P],
                        pst.rearrange("p (a b) -> p a b", a=4),
                    )

            def proj_k(ht, ms):
                """kTf[ht][:, ms*512:(ms+1)*512] from ktsK[ms]."""
                _mark("proj_k")
                pp = ps_sc.tile([P, 1024], F32, tag="sc", name="sc")
                for dc in range(4):
                    nc.tensor.matmul(
                        pp[:, 0:512], wk_sb[:, dc, ht * P:(ht + 1) * P],
                        ktsK[ms][:, dc, :], start=(dc == 0), stop=(dc == 3),
                    )
                nc.vector.tensor_scalar(
                    kTf[ht][:, ms * 512:(ms + 1) * 512],
                    pp[:, 0:512], kb[:, ht:ht + 1], None, ADD,
                )

            def proj_q(ht, ns):
                _mark("proj_q")
                pp = ps_sc.tile([P, 1024], F32, tag="sc", name="sc")
                for dc in range(4):
                    nc.tensor.matmul(
                        pp[:, 0:512], wq_sb[:, dc, ht * P:(ht + 1) * P],
                        ktsQ[ns][:, dc, :], start=(dc == 0), stop=(dc == 3),
                    )
                nc.vector.tensor_scalar(
                    qTf[ht][:, ns * 512:(ns + 1) * 512],
                    pp[:, 0:512], qb[:, ht:ht + 1], None, ADD,
                )

            def load_vt(msb, dc, eng):
                """DMA-transpose V[msb*512:(msb+1)*512, dc*128:(dc+1)*128]
                into VT_sb[:, dc, msb*512:(msb+1)*512]."""
                eng.dma_start(
                    VT_sb[:, dc, msb * 512:(msb + 1) * 512],
                    d_v[msb * 512:(msb + 1) * 512, dc * P:(dc + 1) * P],
                    transpose=True)

            def emit_v_tile(mt):
                """Project one (DMA-transposed) V m-tile into vha (+v-bias)."""
                _mark("v_tile")
                pp = ps_wk.tile([P, DM], F32, tag="pj", name="pj")
                for dc in range(4):
                    nc.tensor.matmul(
                        pp[:], VT_sb[:, dc, mt * P:(mt + 1) * P],
                        wv_sb[:, dc, :],
                        start=(dc == 0), stop=(dc == 3),
                    )
                nc.vector.tensor_tensor(
                    vha[:, mt, :, 0:64],
                    pp.rearrange("p (h c) -> p h c", h=H), vbb[:], ADD,
                )

            def init_vbb():
                # vbb = ones-col x vbrow: v-bias broadcast over m partitions
                bb = ps_wk.tile([P, DM], F32, tag="pj", name="pj")
                nc.tensor.matmul(bb[:], ones[0:1, 0:P], vbrow[:],
                                 start=True, stop=True)
                nc.vector.tensor_copy(vbb.rearrange("p a b -> p (a b)"), bb[:])
                # ones column of vha
                nc.vector.tensor_copy(
                    vha[:, :, :, 64:65],
                    ones_f32[:, 0:N_MT * H].rearrange(
                        "p (a h) -> p a h", a=N_MT)[:, :, :, None],
                )

            # recip consts
            from concourse.dve_ops import (
                RECIP_APPROX_FAST_CONSTS, RECIPROCAL_APPROX_FAST)
            _rc = RECIP_APPROX_FAST_CONSTS
            _mh2 = {}
            _finishers = []

            def _flush_finishers():
                while _finishers:
                    _finishers.pop(0)()

            def oh_group(w, g, ex_tiles, pool=None):
                """One oh accumulation group of window w: g = ab*4 + j.
                Accumulates oh[n-block j, 65] over all 16 m-tiles, then
                normalizes into mh2. The mh transpose for ab==1 is deferred
                as a finisher (flushed after later PE work is queued)."""
                hp, nb = w // 2, w % 2
                ab, j = g // 4, g % 4
                _mark(f"oh_w{w}")
                h = 2 * hp + ab
                if pool is None:
                    oh = ps_wk.tile([P, 512], F32, tag="pj", name="pj")
                else:
                    oh = pool.tile([P, 1024], F32, tag="sc", name="sc")
                for mu in range(8):
                    for jj in range(2):
                        mt = 2 * mu + jj
                        nc.tensor.matmul(
                            oh[:, 0:65],
                            ex_tiles[mu][ab][:, jj, j * P:(j + 1) * P],
                            vha[:, mt, h, :],
                            start=(mu == 0 and jj == 0),
                            stop=(mu == 7 and jj == 1),
                        )
                rr = nm.tile([P, 1], F32, tag="rr", name="rr")
                nc.vector._custom_dve(
                    RECIPROCAL_APPROX_FAST, out=rr[:], in0=oh[:, 64:65],
                    s0=_rc["s0"], s1=_rc["s1"], imm2=_rc["imm2"],
                )
                if ab == 0:
                    _mh2[j] = mh2_pool.tile([P, 2, DH], BF16, tag=f"mh2_{j}",
                                            name=f"mh2_{j}")
                mh2 = _mh2[j]
                nc.vector.tensor_scalar(
                    mh2[:, ab, :], oh[:, 0:64], rr[:, 0:1], None, MULT,
                )
                _flush_finishers()
                if ab == 1:
                    def fin(mh2=mh2, nb=nb, hp=hp, j=j):
                        _mark(f"oh_w{w}")
                        mtp = ps_wk.tile([P, 512], F32, tag="pj",
                                         name="pj").bitcast(BF16)[:, 0:P]
                        nc.tensor.transpose(
                            mtp, mh2.rearrange("p a b -> p (a b)"), identb[:])
                        nc.vector.tensor_copy(
                            mhT[nb][hp][:, j * P:(j + 1) * P], mtp)
                    _finishers.append(fin)

            def emit_out_group(nt):
                """Output projection for global n-tile nt."""
                _flush_finishers()
                _mark("out_proj")
                nb, jl = nt // 4, nt % 4
                po = ps_wk.tile([P, DO], F32, tag="pj", name="pj")
                for hp in range(4):
                    nc.tensor.matmul(
                        po[:], mhT[nb][hp][:, jl * P:(jl + 1) * P],
                        wp_sb[:, hp, :],
                        start=(hp == 0), stop=False, skip_group_check=True,
                    )
                nc.tensor.matmul(
                    po[:], ones[0:1, 0:P], pb[:],
                    start=False, stop=True, skip_group_check=True,
                )
                ot = nm.tile([P, DO], F32, tag="ot", name="ot")
                nc.vector.tensor_copy(ot[:], po[:])
                (nc.gpsimd if nt % 2 else nc.sync).dma_start(
                    d_out[nt * P:(nt + 1) * P, :], ot[:])

            # === lead-in: first K/Q groups + head-0 projections ===
            kp = {0: load_pair(d_k, 0, nc.sync)}
            load_w_half(wk_sb, d_wk, 0, nc.sync)
            kp[1] = load_pair(d_k, 2, nc.gpsimd)
            load_w_half(wk_sb, d_wk, 1, nc.gpsimd)
            qp = {0: load_pair(d_q, 0, nc.sync),
                  1: load_pair(d_q, 2, nc.gpsimd)}
            load_w_half(wq_sb, d_wq, 0, nc.sync)
            load_w_half(wq_sb, d_wq, 1, nc.gpsimd)
            nc.gpsimd.dma_start(vbrow[:], d_vbrow[:])
            transpose_pair(kp[0], ktsK[0], 0)
            transpose_pair(kp[1], ktsK[0], 2)
            kp[2] = load_pair(d_k, 4, nc.sync)
            kp[3] = load_pair(d_k, 6, nc.gpsimd)
            proj_k(0, 0)
            transpose_pair(qp[0], ktsQ[0], 0)
            transpose_pair(qp[1], ktsQ[0], 2)
            proj_q(0, 0)

            # window-0 filler schedule: (what, arg) per step
            w0_fill = [
                ("kT", (2, 1, 0)), ("kTp", (3, 1, 2)),   # ktsK[1] + projk(0,1)
                ("kT", (4, 2, 0)), ("kTp", (5, 2, 2)),
                ("kT", (6, 3, 0)), ("kTp", (7, 3, 2)),
                ("qT", (2, 1, 0)), ("qTp", (3, 1, 2)),
                ("v", 0), ("v", 1), ("v", 2), ("v", 3),
                ("v", 4), ("v", 5), ("v", 6), ("v", 7),
            ]
            # DMA issue schedule for w0 (step -> list of loads)
            w0_loads = {
                0: [("k", 4), ("k", 5), ("wv", 0), ("wv", 1)],
                1: [("k", 6), ("k", 7), ("vt", 0), ("vt", 1)],
                2: [("q", 2), ("q", 3), ("vt", 2), ("vt", 3)],
                3: [("vt", 4), ("vt", 5), ("vt", 6), ("vt", 7)],
                4: [("vt", 8), ("vt", 9), ("vt", 10), ("vt", 11)],
                5: [("vt", 12), ("vt", 13), ("vt", 14), ("vt", 15)],
                10: [("wp", 0)], 11: [("wp", 1), ("pbl", 0)],
            }

            def do_load(kind, a):
                if kind == "k":
                    kp[a] = load_pair(d_k, 2 * a, nc.gpsimd)
                elif kind == "q":
                    qp[a] = load_pair(d_q, 2 * a, nc.gpsimd)
                elif kind == "vt":
                    msb, dc = a // 4, a % 4
                    load_vt(msb, dc, nc.sync)
                elif kind == "wv":
                    load_w_half(wv_sb, d_wv, a, nc.gpsimd)
                elif kind == "wq":
                    load_w_half(wq_sb, d_wq, a, nc.gpsimd if a % 2 else nc.sync)
                elif kind == "wp":
                    nc.gpsimd.dma_start(
                        wp_sb[:, 2 * a:2 * a + 2, :],
                        d_wp[2 * a * P:(2 * a + 2) * P, :].rearrange(
                            "(j p) c -> p j c", p=P))
                elif kind == "pbl":
                    nc.gpsimd.dma_start(pb[:], d_pb[:])

            def w0_filler(step):
                for kind, a in w0_loads.get(step, []):
                    do_load(kind, a)
                what, arg = w0_fill[step]
                if what == "kT":
                    g, ms, j0 = arg
                    transpose_pair(kp[g], ktsK[ms], j0, dve_only=True)
                elif what == "kTp":
                    g, ms, j0 = arg
                    transpose_pair(kp[g], ktsK[ms], j0, dve_only=True)
                    proj_k(0, ms)
                elif what == "qT":
                    g, ns, j0 = arg
                    transpose_pair(qp[g], ktsQ[ns], j0, dve_only=True)
                elif what == "qTp":
                    g, ns, j0 = arg
                    transpose_pair(qp[g], ktsQ[ns], j0, dve_only=True)
                    proj_q(0, ns)
                    init_vbb()
                elif what == "v":
                    emit_v_tile(arg)

            # === attention windows ===
            prev_ex = None
            for hp in range(4):
                for nb in range(2):
                    w = hp * 2 + nb
                    ex_tiles = [[None, None] for _ in range(8)]
                    ab_order = ([(mu, ab) for mu in range(8) for ab in range(2)]
                                if w < 7 else
                                [(mu, ab) for ab in range(2) for mu in range(8)])
                    for step, (mu, ab) in enumerate(ab_order):
                        _mark(f"scores_w{w}")
                        base = ab * 64
                        sc = ps_sc.tile([P, 1024], F32, tag="sc", name="sc")
                        for jj in range(2):
                            mt = 2 * mu + jj
                            nc.tensor.matmul(
                                sc[:, jj * 512:(jj + 1) * 512],
                                kTf[hp][base:base + 64, mt * P:(mt + 1) * P],
                                qTf[hp][base:base + 64,
                                        nb * 512:(nb + 1) * 512],
                                start=True, stop=True,
                                tile_position=(base, 0),
                            )
                        ex = ex_pool.tile([P, 2, 512], BF16, tag="ex",
                                          name="ex")
                        nc.scalar.activation(
                            ex.rearrange("p a b -> p (a b)"), sc[:], EXP)
                        ex_tiles[mu][ab] = ex
                        # interleaved PE filler work, one unit per step
                        if w == 0:
                            w0_filler(step)
                        elif w == 1:
                            if step < 8:
                                emit_v_tile(8 + step)
                                if step % 2 == 1:
                                    proj_k(1, step // 2)
                                elif step in (2, 6):
                                    proj_q(1, step // 4)
                            else:
                                oh_group(0, step - 8, prev_ex)
                        elif w in (2, 3):
                            ht = w
                            if step % 4 == 0:
                                proj_k(ht, step // 4)
                            elif step % 8 == 1:
                                proj_q(ht, step // 8)
                            if step % 2 == 1:
                                oh_group(w - 1, step // 2, prev_ex)
                        elif w in (4, 5, 6):
                            if step % 2 == 1:
                                oh_group(w - 1, step // 2, prev_ex)
                        elif w == 7:
                            # ab0 phase (steps 0-7): window-6 oh groups
                            # ab1 phase (steps 8-15): nb0 outs + w7 ab0 oh
                            if step < 8:
                                oh_group(6, step, prev_ex)
                            elif step < 12:
                                emit_out_group(step - 8)
                            else:
                                oh_group(7, step - 12, ex_tiles)
                    prev_ex = ex_tiles

            # === tail: window 7 ab1 oh groups + nb1 output projections ===
            oh_group(7, 4, prev_ex, pool=ps_sc)
            oh_group(7, 5, prev_ex, pool=ps_sc)
            emit_out_group(4)
            oh_group(7, 6, prev_ex, pool=ps_sc)
            emit_out_group(5)
            oh_group(7, 7, prev_ex, pool=ps_sc)
            emit_out_group(6)
            emit_out_group(7)

    nc.compile()
    return nc


def kernel(query, key, value, query_kernel, key_kernel, value_kernel,
           projection_kernel, q_bias, k_bias, v_bias, projection_bias):
    query = np.ascontiguousarray(
        np.asarray(query, dtype=np.float32).astype(ml_dtypes.bfloat16))
    key = np.ascontiguousarray(
        np.asarray(key, dtype=np.float32).astype(ml_dtypes.bfloat16))
    value = np.ascontiguousarray(
        np.asarray(value, dtype=np.float32).astype(ml_dtypes.bfloat16))
    scale = np.float32(1.0 / 8.0)  # 1/sqrt(DH)

    wq = np.ascontiguousarray(
        (np.asarray(query_kernel, np.float32) * scale).transpose(1, 0, 2).reshape(DM, HDH)
    ).astype(ml_dtypes.bfloat16)
    wk = np.ascontiguousarray(
        np.asarray(key_kernel, np.float32).transpose(1, 0, 2).reshape(DM, HDH)
    ).astype(ml_dtypes.bfloat16)
    wv = np.ascontiguousarray(
        np.asarray(value_kernel, np.float32).transpose(1, 0, 2).reshape(DM, HDH)
    ).astype(ml_dtypes.bfloat16)
    wp = np.ascontiguousarray(
        np.asarray(projection_kernel, np.float32).reshape(HDH, DO)
    ).astype(ml_dtypes.bfloat16)
    qb = np.ascontiguousarray(
        (np.asarray(q_bias, np.float32) * scale).reshape(HDH).reshape(4, P).T)
    kb = np.ascontiguousarray(np.asarray(k_bias, np.float32).reshape(HDH).reshape(4, P).T)
    vbrow = np.ascontiguousarray(np.asarray(v_bias, np.float32).reshape(1, HDH))
    pb = np.ascontiguousarray(np.asarray(projection_bias, np.float32).reshape(1, DO))

    ident = np.eye(P, dtype=np.float32)
    identb = np.eye(P, dtype=ml_dtypes.bfloat16)
    ones = np.ones((P, P), dtype=np.float32)
    consts = np.ascontiguousarray(np.concatenate([
        ident, ones,
        qb.astype(np.float32), kb.astype(np.float32),
    ], axis=1))

    if "nc" not in _CACHED:
        _CACHED["nc"] = _build()
    nc = _CACHED["nc"]

    shared = dict(wq=wq, wk=wk, wv=wv, wp=wp, consts=consts, identb=identb,
                  vbrow=vbrow, pb=pb)
    in_maps = []
    for c in range(8):
        b, half = c // 2, c % 2
        in_maps.append(dict(
            q=np.ascontiguousarray(query[b, half * NB:(half + 1) * NB, :]),
            k=key[b], v=value[b], **shared))

    trace = os.environ.get("KERNEL_TRACE", "0") == "1"
    try:
        res = run_bass_kernel_spmd(nc, in_maps, core_ids=list(range(8)), trace=trace)
    except ModuleNotFoundError:
        res = run_bass_kernel_spmd(nc, in_maps, core_ids=list(range(8)), trace=False)
    global LAST_EXEC_NS
    LAST_EXEC_NS = res.exec_time_ns
    if trace and res.exec_time_ns is not None:
        print(f"HW exec time: {res.exec_time_ns} ns")
        if res.instructions_and_trace is not None:
            print(f"trace: {res.instructions_and_trace[1]}")

    B = query.shape[0]
    out = np.empty((B, 2 * NB, DO), dtype=np.float32)
    for c in range(8):
        b, half = c // 2, c % 2
        out[b, half * NB:(half + 1) * NB, :] = res.results[c]["out"]
    return out


# revision 28
# speedup vs baseline: 1.2776x; 1.2776x over previous
"""Multi-head attention Trainium2 Bass kernel.

Problem: B=4, N=M=2048, DM=512, H=8, DH=64, DO=512, fp32.
Sharding: 8 cores = (batch b, row-half) -- each core computes full attention
for 1024 query rows of one batch. No collectives.

Per-core dataflow (v4 -- oh flipped to [n, 65], bf16 attention operands,
dense PE schedule):
  - inputs stream in as 2-row-tile pair DMAs alternating the SP(HWDGE) and
    Pool(SWDGE) queues; constants arrive as one packed byte-tensor DMA
  - PE-transpose Q,K,V 128x128 blocks; transposed K/Q staging persists so
    per-head projections interleave into later attention windows
  - kTf/qTf [hdh, m|n] bf16 (bias + 1/sqrt(dh) folded host-side)
  - vha [m, h, 65] bf16 = [Vh + vb | 1]  (v-bias exact since sum(attn)=1)
  - scoresT[m, n] = kh @ qhT per head pair (tile_position row packing);
    window 0 scores interleave with the K/Q lead-in so exp starts early
  - exp on ScalarE (PSUM fp32 -> SBUF bf16)
  - oh[n, 65] = ex^T(stationary) @ vha(moving, F=65); col 64 = denominator
  - normalize on DVE: per-partition reciprocal + multiply -> mh2 bf16;
    mh transposes are deferred "finishers" so the PE never waits on DVE
  - out[n, do] = sum_hp mhT_hp^T @ wp_hp + bias (ones-row matmul)
Loop nest: hp (head pair) outer, nb (n-half) inner; window w = hp*2+nb.
V projection fills windows 0-1 (oh of w0 runs late in w1); kTf/qTf
head-pair projections fill windows 1-3; nb0 output projections fill
window 7 (ab-major scores there); nb1 outputs drain in the tail.
"""
import os
import sys

sys.path.insert(0, "/opt/trn_rl_repo")

import numpy as np
import ml_dtypes

import concourse.bass as bass
import concourse.mybir as mybir
import concourse.tile as tile
from concourse import bacc
from concourse.bass_utils import run_bass_kernel_spmd

F32 = mybir.dt.float32
F32R = mybir.dt.float32r
BF16 = mybir.dt.bfloat16
U8 = mybir.dt.uint8
EXP = mybir.ActivationFunctionType.Exp
ADD = mybir.AluOpType.add
MULT = mybir.AluOpType.mult

P = 128
DM = 512
HDH = 512
DH = 64
H = 8
NB = 1024     # query rows per core
M = 2048      # kv rows
DO = 512
N_MT = M // P
N_QT = NB // P

_CACHED = {}
LAST_EXEC_NS = None
_SECTION = None  # optional trace-attribution hook: list whose [0] is set


def _mark(s):
    if _SECTION is not None:
        _SECTION[0] = s


def _build():
    nc = bacc.Bacc("TRN2", target_bir_lowering=False, debug=False)

    d_q = nc.declare_dram_parameter("q", [NB, DM], BF16, isOutput=False)
    d_k = nc.declare_dram_parameter("k", [M, DM], BF16, isOutput=False)
    d_v = nc.declare_dram_parameter("v", [M, DM], BF16, isOutput=False)
    d_wq = nc.declare_dram_parameter("wq", [DM, HDH], BF16, isOutput=False)
    d_wk = nc.declare_dram_parameter("wk", [DM, HDH], BF16, isOutput=False)
    d_wv = nc.declare_dram_parameter("wv", [DM, HDH], BF16, isOutput=False)
    d_wp = nc.declare_dram_parameter("wp", [HDH, DO], BF16, isOutput=False)
    d_consts = nc.declare_dram_parameter("consts", [P, 264], F32R, isOutput=False)
    d_idb = nc.declare_dram_parameter("identb", [P, P], BF16, isOutput=False)
    d_vbrow = nc.declare_dram_parameter("vbrow", [1, HDH], F32R, isOutput=False)
    d_pb = nc.declare_dram_parameter("pb", [1, DO], F32R, isOutput=False)
    d_out = nc.declare_dram_parameter("out", [NB, DO], F32, isOutput=True)

    with tile.TileContext(nc) as tc:
        from contextlib import ExitStack
        with ExitStack() as ctx:
            persist = ctx.enter_context(tc.tile_pool(name="persist", bufs=1))
            raw = ctx.enter_context(tc.tile_pool(name="raw", bufs=8))
            vtt_pool = ctx.enter_context(tc.tile_pool(name="vtt", bufs=3))
            ex_pool = ctx.enter_context(tc.tile_pool(name="expp", bufs=24))
            nm = ctx.enter_context(tc.tile_pool(name="nm", bufs=4))
            mh2_pool = ctx.enter_context(tc.tile_pool(name="mh2", bufs=3))
            ps_sc = ctx.enter_context(tc.tile_pool(name="ps_sc", bufs=3, space="PSUM"))
            ps_wk = ctx.enter_context(tc.tile_pool(name="ps_wk", bufs=2, space="PSUM"))

            # --- packed constants: one f32r DMA (+ tiny bf16 identity) ---
            consts = persist.tile([P, 264], F32R, tag="consts", name="consts")
            nc.sync.dma_start(consts[:], d_consts[:])
            identb = persist.tile([P, P], BF16, tag="identb", name="identb")
            nc.sync.dma_start(identb[:], d_idb[:])
            ident = consts[:, 0:128]
            ones = consts[:, 128:256]
            ones_f32 = consts[:, 128:256].bitcast(F32)
            qb = consts[:, 256:260].bitcast(F32)
            kb = consts[:, 260:264].bitcast(F32)

            # --- persistent tensors ---
            kTf = [persist.tile([P, M], BF16, tag=f"kTf{i}", name=f"kTf{i}")
                   for i in range(4)]
            qTf = [persist.tile([P, NB], BF16, tag=f"qTf{i}", name=f"qTf{i}")
                   for i in range(4)]
            ktsK = [persist.tile([P, 4, 512], BF16, tag=f"ktsK{i}", name=f"ktsK{i}")
                    for i in range(4)]
            ktsQ = [persist.tile([P, 4, 512], BF16, tag=f"ktsQ{i}", name=f"ktsQ{i}")
                    for i in range(2)]
            vha = persist.tile([P, N_MT, H, 65], BF16, tag="vha", name="vha")
            mhT = [[persist.tile([P, 512], BF16, tag=f"mhT{nb}_{hp}",
                                 name=f"mhT{nb}_{hp}")
                    for hp in range(4)] for nb in range(2)]
            vbb = persist.tile([P, H, DH], BF16, tag="vbb", name="vbb")
            pb = persist.tile([1, DO], F32R, tag="pb", name="pb")
            vbrow = persist.tile([1, HDH], F32R, tag="vbrow", name="vbrow")
            wk_sb = persist.tile([P, 4, HDH], BF16, tag="wk", name="wk")
            wq_sb = persist.tile([P, 4, HDH], BF16, tag="wq", name="wq")
            wv_sb = persist.tile([P, 4, HDH], BF16, tag="wv", name="wv")
            wp_sb = persist.tile([P, 4, DO], BF16, tag="wp", name="wp")

            def load_pair(d_src, t0, eng):
                """One DMA loading 2 bf16 row-tiles as [p, j, c]."""
                st = raw.tile([P, 2, DM], BF16, tag="pairb", name="pairb")
                eng.dma_start(
                    st[:],
                    d_src[t0 * P:(t0 + 2) * P, :].rearrange(
                        "(j p) c -> p j c", p=P))
                return st

            def load_w_half(w_sb, d_w, h, eng):
                eng.dma_start(
                    w_sb[:, 2 * h:2 * h + 2, :],
                    d_w[2 * h * P:(2 * h + 2) * P, :].rearrange(
                        "(j p) c -> p j c", p=P))

            def transpose_pair(st, ts, j0, dve_only=False):
                """Transpose 2 row-tiles from st into ts slices j0, j0+1."""
                _mark("in_transpose")
                for jj in range(2):
                    pst = ps_wk.tile([P, DM], F32, tag="pj",
                                     name="pj").bitcast(BF16)[:, 0:DM]
                    for dc in range(4):
                        nc.tensor.transpose(
                            pst[:, dc * P:(dc + 1) * P],
                            st[:, jj, dc * P:(dc + 1) * P], identb[:],
                        )
                    eng = (nc.vector.tensor_copy if (dve_only or jj % 2)
                           else nc.scalar.copy)
                    eng(
                        ts[:, :, (j0 + jj) * P:(j0 + jj + 1) * P],
                        pst.rearrange("p (a b) -> p a b", a=4),
                    )

            def proj_k(ht, ms):
                """kTf[ht][:, ms*512:(ms+1)*512] from ktsK[ms]."""
                _mark("proj_k")
                pp = ps_sc.tile([P, 1024], F32, tag="sc", name="sc")
                for dc in range(4):
                    nc.tensor.matmul(
                        pp[:, 0:512], wk_sb[:, dc, ht * P:(ht + 1) * P],
                        ktsK[ms][:, dc, :], start=(dc == 0), stop=(dc == 3),
                    )
                nc.vector.tensor_scalar(
                    kTf[ht][:, ms * 512:(ms + 1) * 512],
                    pp[:, 0:512], kb[:, ht:ht + 1], None, ADD,
                )

            def proj_q(ht, ns):
                _mark("proj_q")
                pp = ps_sc.tile([P, 1024], F32, tag="sc", name="sc")
                for dc in range(4):
                    nc.tensor.matmul(
                        pp[:, 0:512], wq_sb[:, dc, ht * P:(ht + 1) * P],
                        ktsQ[ns][:, dc, :], start=(dc == 0), stop=(dc == 3),
                    )
                nc.vector.tensor_scalar(
                    qTf[ht][:, ns * 512:(ns + 1) * 512],
                    pp[:, 0:512], qb[:, ht:ht + 1], None, ADD,
                )

            _vpairs = {}

            def load_v_pair(g):
                _vpairs[g] = load_pair(d_v, 2 * g,
                                       nc.gpsimd if g % 2 else nc.sync)

            def emit_v_tile(mt):
                """Transpose + project one V m-tile into vha (+v-bias)."""
                _mark("v_tile")
                vn = _vpairs[mt // 2][:, mt % 2, :]
                pst = ps_wk.tile([P, DM], F32, tag="pj",
                                 name="pj").bitcast(BF16)[:, 0:DM]
                for dc in range(4):
                    nc.tensor.transpose(
                        pst[:, dc * P:(dc + 1) * P], vn[:, dc * P:(dc + 1) * P],
                        identb[:],
                    )
                vtt = vtt_pool.tile([P, 4, P], BF16, tag="vtt", name="vtt")
                nc.vector.tensor_copy(vtt[:], pst.rearrange("p (a b) -> p a b", a=4))
                pp = ps_wk.tile([P, DM], F32, tag="pj", name="pj")
                for dc in range(4):
                    nc.tensor.matmul(
                        pp[:], vtt[:, dc, :], wv_sb[:, dc, :],
                        start=(dc == 0), stop=(dc == 3),
                    )
                nc.vector.tensor_tensor(
                    vha[:, mt, :, 0:64],
                    pp.rearrange("p (h c) -> p h c", h=H), vbb[:], ADD,
                )

            def init_vbb():
                # vbb = ones-col x vbrow: v-bias broadcast over m partitions
                bb = ps_wk.tile([P, DM], F32, tag="pj", name="pj")
                nc.tensor.matmul(bb[:], ones[0:1, 0:P], vbrow[:],
                                 start=True, stop=True)
                nc.vector.tensor_copy(vbb.rearrange("p a b -> p (a b)"), bb[:])
                # ones column of vha
                nc.vector.tensor_copy(
                    vha[:, :, :, 64:65],
                    ones_f32[:, 0:N_MT * H].rearrange(
                        "p (a h) -> p a h", a=N_MT)[:, :, :, None],
                )

            # recip consts
            from concourse.dve_ops import (
                RECIP_APPROX_FAST_CONSTS, RECIPROCAL_APPROX_FAST)
            _rc = RECIP_APPROX_FAST_CONSTS
            _mh2 = {}
            _finishers = []

            def _flush_finishers():
                while _finishers:
                    _finishers.pop(0)()

            def oh_group(w, g, ex_tiles, pool=None):
                """One oh accumulation group of window w: g = ab*4 + j.
                Accumulates oh[n-block j, 65] over all 16 m-tiles, then
                normalizes into mh2. The mh transpose for ab==1 is deferred
                as a finisher (flushed after later PE work is queued)."""
                hp, nb = w // 2, w % 2
                ab, j = g // 4, g % 4
                _mark(f"oh_w{w}")
                h = 2 * hp + ab
                if pool is None:
                    oh = ps_wk.tile([P, 512], F32, tag="pj", name="pj")
                else:
                    oh = pool.tile([P, 1024], F32, tag="sc", name="sc")
                for mu in range(8):
                    for jj in range(2):
                        mt = 2 * mu + jj
                        nc.tensor.matmul(
                            oh[:, 0:65],
                            ex_tiles[mu][ab][:, jj, j * P:(j + 1) * P],
                            vha[:, mt, h, :],
                            start=(mu == 0 and jj == 0),
                            stop=(mu == 7 and jj == 1),
                        )
                rr = nm.tile([P, 1], F32, tag="rr", name="rr")
                nc.vector._custom_dve(
                    RECIPROCAL_APPROX_FAST, out=rr[:], in0=oh[:, 64:65],
                    s0=_rc["s0"], s1=_rc["s1"], imm2=_rc["imm2"],
                )
                if ab == 0:
                    _mh2[j] = mh2_pool.tile([P, 2, DH], BF16, tag=f"mh2_{j}",
                                            name=f"mh2_{j}")
                mh2 = _mh2[j]
                nc.vector.tensor_scalar(
                    mh2[:, ab, :], oh[:, 0:64], rr[:, 0:1], None, MULT,
                )
                _flush_finishers()
                if ab == 1:
                    def fin(mh2=mh2, nb=nb, hp=hp, j=j):
                        _mark(f"oh_w{w}")
                        mtp = ps_wk.tile([P, 512], F32, tag="pj",
                                         name="pj").bitcast(BF16)[:, 0:P]
                        nc.tensor.transpose(
                            mtp, mh2.rearrange("p a b -> p (a b)"), identb[:])
                        nc.vector.tensor_copy(
                            mhT[nb][hp][:, j * P:(j + 1) * P], mtp)
                    _finishers.append(fin)

            def emit_out_group(nt):
                """Output projection for global n-tile nt."""
                _flush_finishers()
                _mark("out_proj")
                nb, jl = nt // 4, nt % 4
                po = ps_wk.tile([P, DO], F32, tag="pj", name="pj")
                for hp in range(4):
                    nc.tensor.matmul(
                        po[:], mhT[nb][hp][:, jl * P:(jl + 1) * P],
                        wp_sb[:, hp, :],
                        start=(hp == 0), stop=False, skip_group_check=True,
                    )
                nc.tensor.matmul(
                    po[:], ones[0:1, 0:P], pb[:],
                    start=False, stop=True, skip_group_check=True,
                )
                ot = nm.tile([P, DO], F32, tag="ot", name="ot")
                nc.vector.tensor_copy(ot[:], po[:])
                (nc.gpsimd if nt % 2 else nc.sync).dma_start(
                    d_out[nt * P:(nt + 1) * P, :], ot[:])

            # === lead-in: first K/Q groups + head-0 projections ===
            kp = {0: load_pair(d_k, 0, nc.sync)}
            load_w_half(wk_sb, d_wk, 0, nc.sync)
            kp[1] = load_pair(d_k, 2, nc.gpsimd)
            load_w_half(wk_sb, d_wk, 1, nc.gpsimd)
            qp = {0: load_pair(d_q, 0, nc.sync),
                  1: load_pair(d_q, 2, nc.gpsimd)}
            load_w_half(wq_sb, d_wq, 0, nc.sync)
            load_w_half(wq_sb, d_wq, 1, nc.gpsimd)
            nc.gpsimd.dma_start(vbrow[:], d_vbrow[:])
            transpose_pair(kp[0], ktsK[0], 0)
            transpose_pair(kp[1], ktsK[0], 2)
            kp[2] = load_pair(d_k, 4, nc.sync)
            kp[3] = load_pair(d_k, 6, nc.gpsimd)
            proj_k(0, 0)
            transpose_pair(qp[0], ktsQ[0], 0)
            transpose_pair(qp[1], ktsQ[0], 2)
            proj_q(0, 0)

            # window-0 filler schedule: (what, arg) per step
            w0_fill = [
                ("kT", (2, 1, 0)), ("kTp", (3, 1, 2)),   # ktsK[1] + projk(0,1)
                ("kT", (4, 2, 0)), ("kTp", (5, 2, 2)),
                ("kT", (6, 3, 0)), ("kTp", (7, 3, 2)),
                ("qT", (2, 1, 0)), ("qTp", (3, 1, 2)),
                ("v", 0), ("v", 1), ("v", 2), ("v", 3),
                ("v", 4), ("v", 5), ("v", 6), ("v", 7),
            ]
            # DMA issue schedule for w0 (step -> list of loads)
            w0_loads = {
                0: [("k", 4), ("k", 5)], 1: [("k", 6), ("k", 7)],
                2: [("q", 2), ("q", 3)],
                3: [("wv", 0), ("wv", 1)],
                4: [("v", 0), ("v", 1)], 5: [("v", 2), ("v", 3)],
                6: [("v", 4), ("v", 5)], 7: [("v", 6), ("v", 7)],
                10: [("wp", 0)], 11: [("wp", 1), ("pbl", 0)],
            }

            def do_load(kind, a):
                if kind == "k":
                    kp[a] = load_pair(d_k, 2 * a, nc.gpsimd if a % 2 else nc.sync)
                elif kind == "q":
                    qp[a] = load_pair(d_q, 2 * a,
                                      nc.gpsimd if a % 2 else nc.sync)
                elif kind == "v":
                    load_v_pair(a)
                elif kind == "wv":
                    load_w_half(wv_sb, d_wv, a, nc.gpsimd if a % 2 else nc.sync)
                elif kind == "wq":
                    load_w_half(wq_sb, d_wq, a, nc.gpsimd if a % 2 else nc.sync)
                elif kind == "wp":
                    nc.gpsimd.dma_start(
                        wp_sb[:, 2 * a:2 * a + 2, :],
                        d_wp[2 * a * P:(2 * a + 2) * P, :].rearrange(
                            "(j p) c -> p j c", p=P))
                elif kind == "pbl":
                    nc.gpsimd.dma_start(pb[:], d_pb[:])

            def w0_filler(step):
                for kind, a in w0_loads.get(step, []):
                    do_load(kind, a)
                what, arg = w0_fill[step]
                if what == "kT":
                    g, ms, j0 = arg
                    transpose_pair(kp[g], ktsK[ms], j0, dve_only=True)
                elif what == "kTp":
                    g, ms, j0 = arg
                    transpose_pair(kp[g], ktsK[ms], j0, dve_only=True)
                    proj_k(0, ms)
                elif what == "qT":
                    g, ns, j0 = arg
                    transpose_pair(qp[g], ktsQ[ns], j0, dve_only=True)
                elif what == "qTp":
                    g, ns, j0 = arg
                    transpose_pair(qp[g], ktsQ[ns], j0, dve_only=True)
                    proj_q(0, ns)
                    init_vbb()
                elif what == "v":
                    emit_v_tile(arg)

            # === attention windows ===
            prev_ex = None
            for hp in range(4):
                for nb in range(2):
                    w = hp * 2 + nb
                    ex_tiles = [[None, None] for _ in range(8)]
                    ab_order = ([(mu, ab) for mu in range(8) for ab in range(2)]
                                if w < 7 else
                                [(mu, ab) for ab in range(2) for mu in range(8)])
                    for step, (mu, ab) in enumerate(ab_order):
                        _mark(f"scores_w{w}")
                        base = ab * 64
                        sc = ps_sc.tile([P, 1024], F32, tag="sc", name="sc")
                        for jj in range(2):
                            mt = 2 * mu + jj
                            nc.tensor.matmul(
                                sc[:, jj * 512:(jj + 1) * 512],
                                kTf[hp][base:base + 64, mt * P:(mt + 1) * P],
                                qTf[hp][base:base + 64,
                                        nb * 512:(nb + 1) * 512],
                                start=True, stop=True,
                                tile_position=(base, 0),
                            )
                        ex = ex_pool.tile([P, 2, 512], BF16, tag="ex",
                                          name="ex")
                        nc.scalar.activation(
                            ex.rearrange("p a b -> p (a b)"), sc[:], EXP)
                        ex_tiles[mu][ab] = ex
                        # interleaved PE filler work, one unit per step
                        if w == 0:
                            w0_filler(step)
                        elif w == 1:
                            if step < 8:
                                emit_v_tile(8 + step)
                                if step % 2 == 1:
                                    proj_k(1, step // 2)
                                elif step in (2, 6):
                                    proj_q(1, step // 4)
                            else:
                                oh_group(0, step - 8, prev_ex)
                        elif w == 2:
                            if step in (2, 6):
                                proj_q(2, step // 4)
                            if step % 2 == 1:
                                oh_group(1, step // 2, prev_ex)
                        elif w == 3:
                            if step % 4 == 0:
                                proj_k(2, step // 4)
                            elif step in (2, 6):
                                proj_q(3, step // 4)
                            if step % 2 == 1:
                                oh_group(2, step // 2, prev_ex)
                        elif w in (4, 5):
                            if step in (2, 10):
                                proj_k(3, (w - 4) * 2 + step // 8)
                            if step % 2 == 1:
                                oh_group(w - 1, step // 2, prev_ex)
                        elif w == 6:
                            if step % 2 == 1:
                                oh_group(5, step // 2, prev_ex)
                        elif w == 7:
                            # ab0 phase (steps 0-7): window-6 oh groups
                            # ab1 phase (steps 8-15): nb0 outs + w7 ab0 oh
                            if step < 8:
                                oh_group(6, step, prev_ex)
                            elif step < 12:
                                emit_out_group(step - 8)
                            else:
                                oh_group(7, step - 12, ex_tiles)
                    prev_ex = ex_tiles

            # === tail: window 7 ab1 oh groups + nb1 output projections ===
            oh_group(7, 4, prev_ex, pool=ps_sc)
            oh_group(7, 5, prev_ex, pool=ps_sc)
            emit_out_group(4)
            oh_group(7, 6, prev_ex, pool=ps_sc)
            emit_out_group(5)
            oh_group(7, 7, prev_ex, pool=ps_sc)
            emit_out_group(6)
            emit_out_group(7)

    nc.compile()
    return nc


def kernel(query, key, value, query_kernel, key_kernel, value_kernel,
           projection_kernel, q_bias, k_bias, v_bias, projection_bias):
    query = np.ascontiguousarray(
        np.asarray(query, dtype=np.float32).astype(ml_dtypes.bfloat16))
    key = np.ascontiguousarray(
        np.asarray(key, dtype=np.float32).astype(ml_dtypes.bfloat16))
    value = np.ascontiguousarray(
        np.asarray(value, dtype=np.float32).astype(ml_dtypes.bfloat16))
    scale = np.float32(1.0 / 8.0)  # 1/sqrt(DH)

    wq = np.ascontiguousarray(
        (np.asarray(query_kernel, np.float32) * scale).transpose(1, 0, 2).reshape(DM, HDH)
    ).astype(ml_dtypes.bfloat16)
    wk = np.ascontiguousarray(
        np.asarray(key_kernel, np.float32).transpose(1, 0, 2).reshape(DM, HDH)
    ).astype(ml_dtypes.bfloat16)
    wv = np.ascontiguousarray(
        np.asarray(value_kernel, np.float32).transpose(1, 0, 2).reshape(DM, HDH)
    ).astype(ml_dtypes.bfloat16)
    wp = np.ascontiguousarray(
        np.asarray(projection_kernel, np.float32).reshape(HDH, DO)
    ).astype(ml_dtypes.bfloat16)
    qb = np.ascontiguousarray(
        (np.asarray(q_bias, np.float32) * scale).reshape(HDH).reshape(4, P).T)
    kb = np.ascontiguousarray(np.asarray(k_bias, np.float32).reshape(HDH).reshape(4, P).T)
    vbrow = np.ascontiguousarray(np.asarray(v_bias, np.float32).reshape(1, HDH))
    pb = np.ascontiguousarray(np.asarray(projection_bias, np.float32).reshape(1, DO))

    ident = np.eye(P, dtype=np.float32)
    identb = np.eye(P, dtype=ml_dtypes.bfloat16)
    ones = np.ones((P, P), dtype=np.float32)
    consts = np.ascontiguousarray(np.concatenate([
        ident, ones,
        qb.astype(np.float32), kb.astype(np.float32),
    ], axis=1))

    if "nc" not in _CACHED:
        _CACHED["nc"] = _build()
    nc = _CACHED["nc"]

    shared = dict(wq=wq, wk=wk, wv=wv, wp=wp, consts=consts, identb=identb,
                  vbrow=vbrow, pb=pb)
    in_maps = []
    for c in range(8):
        b, half = c // 2, c % 2
        in_maps.append(dict(
            q=np.ascontiguousarray(query[b, half * NB:(half + 1) * NB, :]),
            k=key[b], v=value[b], **shared))

    trace = os.environ.get("KERNEL_TRACE", "0") == "1"
    try:
        res = run_bass_kernel_spmd(nc, in_maps, core_ids=list(range(8)), trace=trace)
    except ModuleNotFoundError:
        res = run_bass_kernel_spmd(nc, in_maps, core_ids=list(range(8)), trace=False)
    global LAST_EXEC_NS
    LAST_EXEC_NS = res.exec_time_ns
    if trace and res.exec_time_ns is not None:
        print(f"HW exec time: {res.exec_time_ns} ns")
        if res.instructions_and_trace is not None:
            print(f"trace: {res.instructions_and_trace[1]}")

    B = query.shape[0]
    out = np.empty((B, 2 * NB, DO), dtype=np.float32)
    for c in range(8):
        b, half = c // 2, c % 2
        out[b, half * NB:(half + 1) * NB, :] = res.results[c]["out"]
    return out
